# revision 13
# baseline (speedup 1.0000x reference)
"""Single-head causal attention with RoPE on 8 Trainium2 NeuronCores.

Problem: x:(8,2048,1024), Wq/Wk/Wv:(1024,64) -> out:(8,2048,64)
  q = rope(x@Wq); k = rope(x@Wk); v = x@Wv
  out = softmax(causal(q k^T / sqrt(64))) @ v

Sharding: data-parallel over batch B=8, one batch element per core.

Per-core dataflow (transposed [feature, time] layout so the softmax
reduction lands on the PSUM partition dim via a ones-column matmul):
  A(i): qkT = [Wq|Wk]^T @ xT_chunk (PE, M=128); rot = R2 @ bf16(qkT) (PE)
        q'k' = qkT*coscos + rot*sinsin (DVE) -> qkrope bf16 (q rows 0:64,
        k rows 64:128); swapped[k';q'] partition copies via SBUF-SBUF DMA
        v-proj for chunk PAIRS col-tiled: two chunks run CONCURRENTLY in
        PE column groups 0:64 / 64:128
  B(g): v natural via PE transpose -> vnat[128,16,65] (ones col 64)
  C(i): s-blocks in PAIRS: two K=64 score MMs run CONCURRENTLY in PE row
        groups 0:64 / 64:128 (tile_position from base partitions); ONE
        exp ACTIVATE covers both; diagonal blocks paired + causal-trimmed
        [num;den] psum += vnat_sb^T @ pexp (PE, M=65; ones col -> den)
        evict [num;den] to bf16 and DMA out; normalization (num/den and
        the final [h,t]->[t,h] transpose) happens on HOST after gather
  DMA: x-stream has priority on BOTH HWDGE queues (sync + scalar);
  cos/sin loaded as 64 rows and duplicated on-chip; small consts +
  partition dups on gpsimd (SWDGE).
"""

import numpy as np
import ml_dtypes

B, T, C, H = 8, 2048, 1024, 64
NCORES = 8
CHUNK = 512
NCHUNK = T // CHUNK  # 4
NSB = T // 128       # 16 s-blocks
NCB = C // 128       # 8 c-blocks

bf16 = ml_dtypes.bfloat16


# ---------------------------------------------------------------- host consts
def _build_consts():
    half = H // 2
    inv_freq = (1.0 / (10000.0 ** (np.arange(half, dtype=np.float32) / half))).astype(
        np.float32
    )
    t = np.arange(T, dtype=np.float32)
    freqs = t[:, None] * inv_freq[None, :]  # (T, half) fp32
    cos = np.repeat(np.cos(freqs), 2, axis=-1)  # (T, H)
    sin = np.repeat(np.sin(freqs), 2, axis=-1)
    cosT = np.ascontiguousarray(cos.T).astype(bf16)  # (H, T)
    sinT = np.ascontiguousarray(sin.T).astype(bf16)

    # rot = R @ q with rot[2i] = -q[2i+1], rot[2i+1] = q[2i]
    Rm = np.zeros((H, H), np.float32)
    for i in range(half):
        Rm[2 * i, 2 * i + 1] = -1.0
        Rm[2 * i + 1, 2 * i] = 1.0
    r2 = np.zeros((128, 128), np.float32)
    r2[0:H, 0:H] = Rm.T
    r2[H:128, H:128] = Rm.T
    r2 = r2.astype(bf16)

    sl = np.arange(128)
    trimask = (sl[:, None] <= sl[None, :]).astype(bf16)  # (128, 128)

    identb2 = np.concatenate([np.eye(H), np.eye(H)], axis=0).astype(bf16)  # (128, 64)

    # half-swap permutation: swapped[p] = x[(p+64) % 128]; symmetric
    pswap = np.zeros((128, 128), np.float32)
    for p in range(128):
        pswap[p, (p + 64) % 128] = 1.0
    pswap = pswap.astype(bf16)

    coscos = np.ascontiguousarray(np.concatenate([cosT, cosT], axis=0))  # (128, T)
    sinsin = np.ascontiguousarray(np.concatenate([sinT, sinT], axis=0))

    return coscos, sinsin, r2, trimask, identb2, pswap


# ---------------------------------------------------------------- bass program
def _build_bass():
    import concourse.mybir as mybir
    import concourse.tile as tile
    from concourse import bacc
    from concourse.bass import ts

    BF = mybir.dt.bfloat16
    F32 = mybir.dt.float32
    Exp = mybir.ActivationFunctionType.Exp

    nc = bacc.Bacc(
        "TRN2",
        target_bir_lowering=False,
        debug=False,
        enable_asserts=False,
        num_devices=NCORES,
    )

    # xT prepacked on host to SBUF layout [128(p), chunk, cblk, 512]
    xT_d = nc.dram_tensor("xTp", [128, NCHUNK, NCB, CHUNK], BF, kind="ExternalInput")
    wqk_d = nc.dram_tensor("wqkp", [128, NCB, 128], BF, kind="ExternalInput")
    wv_d = nc.dram_tensor("wvp", [128, NCB, H], BF, kind="ExternalInput")
    r2_d = nc.dram_tensor("r2", [128, 128], BF, kind="ExternalInput")
    coscos_d = nc.dram_tensor("coscos", [128, T], BF, kind="ExternalInput")
    sinsin_d = nc.dram_tensor("sinsin", [128, T], BF, kind="ExternalInput")
    trimask_d = nc.dram_tensor("trimask", [128, 128], BF, kind="ExternalInput")
    identb_d = nc.dram_tensor("identb2", [128, H], BF, kind="ExternalInput")
    pswap_d = nc.dram_tensor("pswap", [128, 128], BF, kind="ExternalInput")
    out_d = nc.dram_tensor("outp", [NCHUNK, H + 1, CHUNK], BF, kind="ExternalOutput")

    with tile.TileContext(nc) as tc:
        with (
            tc.tile_pool(name="persist", bufs=1) as persist,
            tc.tile_pool(name="work", bufs=3) as work,
            tc.tile_pool(name="pexpp", bufs=5) as pexpp,
            tc.tile_pool(name="ps_scratch", bufs=2, space="PSUM") as ps_scratch,
            tc.tile_pool(name="ps_sc", bufs=2, space="PSUM") as ps_sc,
            tc.tile_pool(name="ps_out", bufs=2, space="PSUM") as ps_out,
        ):
            # ---- persistent SBUF tensors
            wqk_sb = persist.tile([128, NCB, 128], BF)
            wv_sb = persist.tile([128, NCB, H], BF)
            r2_sb = persist.tile([128, 128], BF)
            coscos_sb = persist.tile([128, T], BF)
            sinsin_sb = persist.tile([128, T], BF)
            trimask_sb = persist.tile([128, 128], BF)
            identb_sb = persist.tile([128, H], BF)
            pswap_sb = persist.tile([128, 128], BF)
            xT_sb = persist.tile([128, NCHUNK, NCB, CHUNK], BF)
            qkrope = persist.tile([128, T], BF)   # q' rows 0:64, k' rows 64:128
            swapped = persist.tile([128, T], BF)  # k' rows 0:64, q' rows 64:128
            vT_sb = persist.tile([128, NCHUNK // 2, CHUNK], BF)  # stacked pairs
            vnat = persist.tile([128, NSB, H + 1], BF)

            # ---- sync HWDGE queue: wqk (gates first matmul), then x stream.
            # chunk 0 is split across BOTH hwdge queues so projections can
            # start as early as possible; consts ordered by need-time.
            nc.sync.dma_start(out=wqk_sb[:], in_=wqk_d.ap())
            nc.sync.dma_start(out=xT_sb[:, 0, 0:4], in_=xT_d.ap()[:, 0, 0:4])
            nc.sync.dma_start(out=coscos_sb[:], in_=coscos_d.ap())
            nc.sync.dma_start(out=sinsin_sb[:], in_=sinsin_d.ap())
            nc.sync.dma_start(out=xT_sb[:, 2], in_=xT_d.ap()[:, 2])
            nc.sync.dma_start(out=xT_sb[:, 3], in_=xT_d.ap()[:, 3])
            # ---- scalar HWDGE queue: rest of chunk 0, small weights, xT1
            nc.scalar.dma_start(out=r2_sb[:], in_=r2_d.ap())
            nc.scalar.dma_start(out=xT_sb[:, 0, 4:8], in_=xT_d.ap()[:, 0, 4:8])
            nc.scalar.dma_start(out=wv_sb[:], in_=wv_d.ap())
            nc.scalar.dma_start(out=xT_sb[:, 1], in_=xT_d.ap()[:, 1])
            # ---- gpsimd (SWDGE) queue: small late-need consts
            nc.gpsimd.dma_start(out=pswap_sb[:], in_=pswap_d.ap())
            nc.gpsimd.dma_start(out=identb_sb[:], in_=identb_d.ap())
            nc.gpsimd.dma_start(out=trimask_sb[:], in_=trimask_d.ap())

            nc.vector.memset(vnat[:], 1.0)  # ones col (64); cols 0:64 overwritten

            # PE warmup: junk matmuls on a zeroed tile bridge the preamble/
            # DMA window so the HAM clock-gate opens before real projections
            zwarm = persist.tile([128, CHUNK], BF)
            nc.vector.memset(zwarm[:], 0.0)
            warm_ps = ps_sc.tile([128, 2, CHUNK], F32, tag="sc", name="warm")
            for w in range(8):
                nc.tensor.matmul(
                    warm_ps[:, 0, :],
                    zwarm[:, 0:128],
                    zwarm[:],
                    start=(w == 0),
                    stop=(w == 7),
                )

            rot_tiles = {}

            def phase_a_proj(i):
                qk_ps = ps_scratch.tile([128, CHUNK], F32, tag="scr", name=f"qk{i}")
                for c in range(NCB):
                    nc.tensor.matmul(
                        qk_ps[:],
                        wqk_sb[:, c, :],
                        xT_sb[:, i, c, :],
                        start=(c == 0),
                        stop=(c == NCB - 1),
                    )
                qkS = work.tile([128, CHUNK], BF, tag="qkS", name=f"qkS{i}")
                nc.vector.tensor_copy(out=qkS[:], in_=qk_ps[:])
                rot_ps = ps_scratch.tile([128, CHUNK], F32, tag="scr", name=f"rot{i}")
                nc.tensor.matmul(rot_ps[:], r2_sb[:], qkS[:], start=True, stop=True)
                rot_tiles[i] = (qkS, rot_ps)

            def phase_a_rope(i):
                tsl = ts(i, CHUNK)
                qkS, rot_ps = rot_tiles[i]
                tmp1 = work.tile([128, CHUNK], BF, tag="tmp1", name=f"t1_{i}")
                nc.vector.tensor_mul(tmp1[:], qkS[:], coscos_sb[:, tsl])
                tmp2 = work.tile([128, CHUNK], BF, tag="tmp2", name=f"t2_{i}")
                nc.vector.tensor_mul(tmp2[:], rot_ps[:], sinsin_sb[:, tsl])
                nc.vector.tensor_add(qkrope[:, tsl], tmp1[:], tmp2[:])
                # partition-swapped copy [k';q'] (PE permutation matmul) so
                # score MMs can run in CONCURRENT PE row groups
                sw_ps = ps_scratch.tile([128, CHUNK], F32, tag="scr", name=f"sw{i}")
                nc.tensor.matmul(
                    sw_ps[:], pswap_sb[:], qkrope[:, tsl], start=True, stop=True
                )
                nc.vector.tensor_copy(out=swapped[:, tsl], in_=sw_ps[:])

            def phase_v(g):
                # v-projection for chunk pair (2g, 2g+1), col-tiled pairs
                # interleaved so both column groups stream concurrently
                i = 2 * g + 1
                v_ps = ps_scratch.tile([128, CHUNK], F32, tag="scr", name=f"v{g}")
                for c in range(NCB):
                    nc.tensor.matmul(
                        v_ps[0:H, :],
                        wv_sb[:, c, :],
                        xT_sb[:, i - 1, c, :],
                        start=(c == 0),
                        stop=(c == NCB - 1),
                        skip_group_check=True,
                    )
                    nc.tensor.matmul(
                        v_ps[H:128, :],
                        wv_sb[:, c, :],
                        xT_sb[:, i, c, :],
                        start=(c == 0),
                        stop=(c == NCB - 1),
                        skip_group_check=True,
                    )
                nc.vector.tensor_copy(out=vT_sb[:, g, :], in_=v_ps[:])

            def phase_b(g):
                # transpose 8 s-blocks of the vT pair group g (chunks 2g, 2g+1)
                for half_ in range(2):
                    vn_ps = ps_out.tile(
                        [128, 4, H], BF, tag="out", name=f"vn{g}_{half_}"
                    )
                    base = H * half_
                    for j in range(4):
                        nc.tensor.transpose(
                            vn_ps[:, j, :],
                            vT_sb[base : base + H, g, ts(j, 128)],
                            identb_sb[base : base + H, :],
                        )
                    first = 8 * g + 4 * half_
                    nc.vector.tensor_copy(
                        out=vnat[:, first : first + 4, 0:H], in_=vn_ps[:]
                    )

            out_tiles = {}

            def phase_c_accum(i):
                nsb = 4 * i + 4  # s-blocks 0 .. 4i+3 (0..4i-1 full, then diag)
                out_ps = ps_out.tile([H + 1, CHUNK], F32, tag="out", name=f"o{i}")
                out_tiles[i] = out_ps

                # units: full pairs then diagonal dual-pairs; software-
                # pipelined so scores(u+1) runs on PE during exp(u) on ACT.
                # each pair's two K=64 score MMs run CONCURRENTLY in PE row
                # groups 0:64 (k' from swapped) and 64:128 (k' from qkrope).
                units = [("pair", p) for p in range(2 * i)] + [
                    ("diag", j) for j in (0, 2)
                ]
                staged = []  # (kind, idx, sc_tile)

                def emit_scores(u):
                    kind, idx = u
                    if kind == "pair":
                        sc2 = ps_sc.tile(
                            [128, 2, CHUNK], F32, tag="sc", name=f"s{i}_{idx}"
                        )
                        sb = 2 * idx
                        nc.tensor.matmul(
                            sc2[:, 0, :],
                            swapped[0:H, ts(sb, 128)],
                            qkrope[0:H, ts(i, CHUNK)],
                            start=True,
                            stop=True,
                            skip_group_check=True,
                        )
                        nc.tensor.matmul(
                            sc2[:, 1, :],
                            qkrope[H:128, ts(sb + 1, 128)],
                            swapped[H:128, ts(i, CHUNK)],
                            start=True,
                            stop=True,
                            skip_group_check=True,
                        )
                    else:
                        j = idx  # diag dual-pair (j, j+1)
                        sc2 = ps_sc.tile(
                            [128, 2, CHUNK], F32, tag="sc", name=f"sd{i}_{j}"
                        )
                        lo0 = 128 * j
                        lo1 = 128 * (j + 1)
                        nc.tensor.matmul(
                            sc2[:, 0, lo0:CHUNK],
                            swapped[0:H, ts(4 * i + j, 128)],
                            qkrope[0:H, i * CHUNK + lo0 : (i + 1) * CHUNK],
                            start=True,
                            stop=True,
                            skip_group_check=True,
                        )
                        nc.tensor.matmul(
                            sc2[:, 1, lo1:CHUNK],
                            qkrope[H:128, ts(4 * i + j + 1, 128)],
                            swapped[H:128, i * CHUNK + lo1 : (i + 1) * CHUNK],
                            start=True,
                            stop=True,
                            skip_group_check=True,
                        )
                    staged.append((kind, idx, sc2))

                def emit_exp_num(stage):
                    kind, idx, sc2 = stage
                    if kind == "pair":
                        pexp2 = pexpp.tile(
                            [128, 2, CHUNK], BF, tag="pexp", name=f"p{i}_{idx}"
                        )
                        nc.scalar.activation(
                            out=pexp2[:], in_=sc2[:], func=Exp, scale=0.125
                        )
                        for h_ in range(2):
                            sb = 2 * idx + h_
                            nc.tensor.matmul(
                                out_ps[:],
                                vnat[:, sb, :],
                                pexp2[:, h_, :],
                                start=(sb == 0),
                                stop=False,
                            )
                    else:
                        j = idx
                        pexp2 = pexpp.tile(
                            [128, 2, CHUNK], BF, tag="pexp", name=f"pd{i}_{j}"
                        )
                        for h_ in range(2):
                            jj = j + h_
                            sb = 4 * i + jj
                            lo = 128 * jj
                            nc.scalar.activation(
                                out=pexp2[:, h_, lo:CHUNK],
                                in_=sc2[:, h_, lo:CHUNK],
                                func=Exp,
                                scale=0.125,
                            )
                            nc.vector.tensor_mul(
                                pexp2[:, h_, lo : lo + 128],
                                pexp2[:, h_, lo : lo + 128],
                                trimask_sb[:],
                            )
                            nc.tensor.matmul(
                                out_ps[:, lo:CHUNK],
                                vnat[:, sb, :],
                                pexp2[:, h_, lo:CHUNK],
                                start=(sb == 0),
                                stop=(sb == nsb - 1),
                            )

                emit_scores(units[0])
                for u in units[1:]:
                    emit_scores(u)
                    emit_exp_num(staged.pop(0))
                emit_exp_num(staged.pop(0))

            def phase_c_out(i):
                # evict [num;den] to bf16 and ship; host divides + transposes
                out_ps = out_tiles[i]
                outb = work.tile([H + 1, CHUNK], BF, tag="outb", name=f"ob{i}")
                nc.vector.tensor_copy(out=outb[:], in_=out_ps[:])
                nc.sync.dma_start(out=out_d.ap()[i], in_=outb[:])

            # emission order == static per-engine FIFO order. Keep long-dep
            # ops (rope needs cos/sin; v needs wv+xT1) from blocking
            # independent work, and emit phase-C scores early so ACT never
            # starves once it gets going.
            phase_a_proj(0)
            phase_a_proj(1)
            phase_a_rope(0)
            phase_a_rope(1)
            phase_v(0)
            phase_b(0)
            phase_c_accum(0)
            phase_a_proj(2)
            phase_a_rope(2)
            phase_c_out(0)
            phase_c_accum(1)
            phase_a_proj(3)
            phase_a_rope(3)
            phase_v(1)
            phase_b(1)
            phase_c_out(1)
            phase_c_accum(2)
            phase_c_out(2)
            phase_c_accum(3)
            phase_c_out(3)

    nc.compile()
    return nc


_NC_CACHE = None


def _get_nc():
    global _NC_CACHE
    if _NC_CACHE is None:
        _NC_CACHE = _build_bass()
    return _NC_CACHE


def make_in_maps(x, Wq, Wk, Wv):
    """Host-side prep: shard over batch + precompute constants."""
    coscos, sinsin, r2, trimask, identb2, pswap = _build_consts()
    wqk = np.concatenate([Wq, Wk], axis=1).astype(bf16)  # (C, 128)
    wv = Wv.astype(bf16)
    wqkp = np.ascontiguousarray(wqk.reshape(NCB, 128, 128).transpose(1, 0, 2))
    wvp = np.ascontiguousarray(wv.reshape(NCB, 128, H).transpose(1, 0, 2))
    in_maps = []
    for b in range(B):
        xT = x[b].T.astype(bf16)  # (C, T)
        xTp = np.ascontiguousarray(
            xT.reshape(NCB, 128, NCHUNK, CHUNK).transpose(1, 2, 0, 3)
        )
        in_maps.append(
            {
                "xTp": xTp,
                "wqkp": wqkp,
                "wvp": wvp,
                "r2": r2,
                "coscos": coscos,
                "sinsin": sinsin,
                "trimask": trimask,
                "identb2": identb2,
                "pswap": pswap,
            }
        )
    return in_maps


def finalize_out(raw):
    """raw: (NCHUNK, H+1, CHUNK) bf16 [num;den] -> (T, H) f32 normalized."""
    r = np.asarray(raw, dtype=np.float32)
    num = r[:, 0:H, :]          # (4, 64, 512)
    den = r[:, H, :]            # (4, 512)
    out = num / den[:, None, :]  # (4, 64, 512)
    return np.ascontiguousarray(out.transpose(0, 2, 1).reshape(T, H))


def kernel(x, Wq, Wk, Wv):
    from concourse.bass_utils import run_bass_kernel_spmd

    x = np.asarray(x, dtype=np.float32)
    Wq = np.asarray(Wq, dtype=np.float32)
    Wk = np.asarray(Wk, dtype=np.float32)
    Wv = np.asarray(Wv, dtype=np.float32)

    nc = _get_nc()
    in_maps = make_in_maps(x, Wq, Wk, Wv)
    res = run_bass_kernel_spmd(nc, in_maps, core_ids=list(range(NCORES)))
    out = np.stack([finalize_out(r["outp"]) for r in res.results])  # (B, T, H)
    return np.ascontiguousarray(out.astype(np.float32))


# revision 16
# speedup vs baseline: 1.0305x; 1.0305x over previous
"""Single-head causal attention with RoPE on 8 Trainium2 NeuronCores.

Problem: x:(8,2048,1024), Wq/Wk/Wv:(1024,64) -> out:(8,2048,64)
  q = rope(x@Wq); k = rope(x@Wk); v = x@Wv
  out = softmax(causal(q k^T / sqrt(64))) @ v

Sharding: data-parallel over batch B=8, one batch element per core.

Per-core dataflow (transposed [feature, time] layout so the softmax
reduction lands on the PSUM partition dim via a ones-column matmul):
  A(i): qkT = [Wq|Wk]^T @ xT_chunk (PE, M=128); rot = R2 @ bf16(qkT) (PE)
        q'k' = qkT*coscos + rot*sinsin (DVE) -> qkrope bf16 (q rows 0:64,
        k rows 64:128); swapped[k';q'] partition copies via SBUF-SBUF DMA
        v-proj for chunk PAIRS col-tiled: two chunks run CONCURRENTLY in
        PE column groups 0:64 / 64:128
  B(g): v natural via PE transpose -> vnat[128,16,65] (ones col 64)
  C(i): s-blocks in PAIRS: two K=64 score MMs run CONCURRENTLY in PE row
        groups 0:64 / 64:128 (tile_position from base partitions); ONE
        exp ACTIVATE covers both; diagonal blocks paired + causal-trimmed
        [num;den] psum += vnat_sb^T @ pexp (PE, M=65; ones col -> den)
        evict [num;den] to bf16 and DMA out; normalization (num/den and
        the final [h,t]->[t,h] transpose) happens on HOST after gather
  DMA: x-stream has priority on BOTH HWDGE queues (sync + scalar);
  cos/sin loaded as 64 rows and duplicated on-chip; small consts +
  partition dups on gpsimd (SWDGE).
"""

import numpy as np
import ml_dtypes

B, T, C, H = 8, 2048, 1024, 64
NCORES = 8
CHUNK = 512
NCHUNK = T // CHUNK  # 4
NSB = T // 128       # 16 s-blocks
NCB = C // 128       # 8 c-blocks

bf16 = ml_dtypes.bfloat16


# ---------------------------------------------------------------- host consts
def _build_consts():
    half = H // 2
    inv_freq = (1.0 / (10000.0 ** (np.arange(half, dtype=np.float32) / half))).astype(
        np.float32
    )
    t = np.arange(T, dtype=np.float32)
    freqs = t[:, None] * inv_freq[None, :]  # (T, half) fp32
    cos = np.repeat(np.cos(freqs), 2, axis=-1)  # (T, H)
    sin = np.repeat(np.sin(freqs), 2, axis=-1)
    cosT = np.ascontiguousarray(cos.T).astype(bf16)  # (H, T)
    sinT = np.ascontiguousarray(sin.T).astype(bf16)

    # rot = R @ q with rot[2i] = -q[2i+1], rot[2i+1] = q[2i]
    Rm = np.zeros((H, H), np.float32)
    for i in range(half):
        Rm[2 * i, 2 * i + 1] = -1.0
        Rm[2 * i + 1, 2 * i] = 1.0
    r2 = np.zeros((128, 128), np.float32)
    r2[0:H, 0:H] = Rm.T
    r2[H:128, H:128] = Rm.T
    r2 = r2.astype(bf16)

    sl = np.arange(128)
    trimask = (sl[:, None] <= sl[None, :]).astype(bf16)  # (128, 128)

    identb2 = np.concatenate([np.eye(H), np.eye(H)], axis=0).astype(bf16)  # (128, 64)

    # half-swap permutation: swapped[p] = x[(p+64) % 128]; symmetric
    pswap = np.zeros((128, 128), np.float32)
    for p in range(128):
        pswap[p, (p + 64) % 128] = 1.0
    pswap = pswap.astype(bf16)

    coscos = np.ascontiguousarray(np.concatenate([cosT, cosT], axis=0))  # (128, T)
    sinsin = np.ascontiguousarray(np.concatenate([sinT, sinT], axis=0))

    return coscos, sinsin, r2, trimask, identb2, pswap


# ---------------------------------------------------------------- bass program
def _build_bass():
    import concourse.mybir as mybir
    import concourse.tile as tile
    from concourse import bacc
    from concourse.bass import ts

    BF = mybir.dt.bfloat16
    F32 = mybir.dt.float32
    Exp = mybir.ActivationFunctionType.Exp

    nc = bacc.Bacc(
        "TRN2",
        target_bir_lowering=False,
        debug=False,
        enable_asserts=False,
        num_devices=NCORES,
    )

    # xT prepacked on host to SBUF layout [128(p), chunk, cblk, 512]
    xT_d = nc.dram_tensor("xTp", [128, NCHUNK, NCB, CHUNK], BF, kind="ExternalInput")
    wqk_d = nc.dram_tensor("wqkp", [128, NCB, 128], BF, kind="ExternalInput")
    wv_d = nc.dram_tensor("wvp", [128, NCB, H], BF, kind="ExternalInput")
    r2_d = nc.dram_tensor("r2", [128, 128], BF, kind="ExternalInput")
    coscos_d = nc.dram_tensor("coscos", [128, T], BF, kind="ExternalInput")
    sinsin_d = nc.dram_tensor("sinsin", [128, T], BF, kind="ExternalInput")
    trimask_d = nc.dram_tensor("trimask", [128, 128], BF, kind="ExternalInput")
    identb_d = nc.dram_tensor("identb2", [128, H], BF, kind="ExternalInput")
    pswap_d = nc.dram_tensor("pswap", [128, 128], BF, kind="ExternalInput")
    out_d = nc.dram_tensor("outp", [NCHUNK, H + 1, CHUNK], BF, kind="ExternalOutput")

    with tile.TileContext(nc) as tc:
        with (
            tc.tile_pool(name="persist", bufs=1) as persist,
            tc.tile_pool(name="work", bufs=3) as work,
            tc.tile_pool(name="pexpp", bufs=5) as pexpp,
            tc.tile_pool(name="ps_scratch", bufs=2, space="PSUM") as ps_scratch,
            tc.tile_pool(name="ps_sc", bufs=2, space="PSUM") as ps_sc,
            tc.tile_pool(name="ps_out", bufs=2, space="PSUM") as ps_out,
        ):
            # ---- persistent SBUF tensors
            wqk_sb = persist.tile([128, NCB, 128], BF)
            wv_sb = persist.tile([128, NCB, H], BF)
            r2_sb = persist.tile([128, 128], BF)
            coscos_sb = persist.tile([128, T], BF)
            sinsin_sb = persist.tile([128, T], BF)
            trimask_sb = persist.tile([128, 128], BF)
            identb_sb = persist.tile([128, H], BF)
            pswap_sb = persist.tile([128, 128], BF)
            xT_sb = persist.tile([128, NCHUNK, NCB, CHUNK], BF)
            qkrope = persist.tile([128, T], BF)   # q' rows 0:64, k' rows 64:128
            swapped = persist.tile([128, T], BF)  # k' rows 0:64, q' rows 64:128
            vT_sb = persist.tile([128, NCHUNK // 2, CHUNK], BF)  # stacked pairs
            vnat = persist.tile([128, NSB, H + 1], BF)

            # ---- sync HWDGE queue: wqk (gates first matmul), then x stream.
            # chunk 0 is split across BOTH hwdge queues so projections can
            # start as early as possible; consts ordered by need-time.
            # ---- sync HWDGE queue (fast): ALL bulk input in strict
            # need-order. xT0 split per c-block-pair so projections start
            # as soon as pieces land; cos/sin split per-chunk so rope(0)
            # isn't gated on the full tables. Scalar HWDGE queue measured
            # ~4x slower -- only small consts go there.
            nc.sync.dma_start(out=wqk_sb[:], in_=wqk_d.ap())
            for p in range(4):
                nc.sync.dma_start(
                    out=xT_sb[:, 0, 2 * p : 2 * p + 2],
                    in_=xT_d.ap()[:, 0, 2 * p : 2 * p + 2],
                )
            nc.sync.dma_start(out=coscos_sb[:, 0:CHUNK], in_=coscos_d.ap()[:, 0:CHUNK])
            nc.sync.dma_start(out=sinsin_sb[:, 0:CHUNK], in_=sinsin_d.ap()[:, 0:CHUNK])
            nc.sync.dma_start(out=xT_sb[:, 1], in_=xT_d.ap()[:, 1])
            nc.sync.dma_start(out=coscos_sb[:, CHUNK:T], in_=coscos_d.ap()[:, CHUNK:T])
            nc.sync.dma_start(out=sinsin_sb[:, CHUNK:T], in_=sinsin_d.ap()[:, CHUNK:T])
            nc.sync.dma_start(out=xT_sb[:, 2], in_=xT_d.ap()[:, 2])
            nc.sync.dma_start(out=xT_sb[:, 3], in_=xT_d.ap()[:, 3])
            # ---- scalar HWDGE queue: small early consts
            nc.scalar.dma_start(out=r2_sb[:], in_=r2_d.ap())
            nc.scalar.dma_start(out=wv_sb[:], in_=wv_d.ap())
            nc.scalar.dma_start(out=pswap_sb[:], in_=pswap_d.ap())
            nc.scalar.dma_start(out=identb_sb[:], in_=identb_d.ap())
            nc.scalar.dma_start(out=trimask_sb[:], in_=trimask_d.ap())

            nc.vector.memset(vnat[:], 1.0)  # ones col (64); cols 0:64 overwritten

            # PE warmup: junk matmuls on a zeroed tile bridge the preamble/
            # DMA window so the HAM clock-gate opens before real projections
            zwarm = persist.tile([128, CHUNK], BF)
            nc.vector.memset(zwarm[:], 0.0)
            warm_ps = ps_sc.tile([128, 2, CHUNK], F32, tag="sc", name="warm")
            for w in range(6):
                nc.tensor.matmul(
                    warm_ps[:, 0, :],
                    zwarm[:, 0:128],
                    zwarm[:],
                    start=(w == 0),
                    stop=(w == 5),
                )

            rot_tiles = {}

            def phase_a_proj(i):
                qk_ps = ps_scratch.tile([128, CHUNK], F32, tag="scr", name=f"qk{i}")
                for c in range(NCB):
                    nc.tensor.matmul(
                        qk_ps[:],
                        wqk_sb[:, c, :],
                        xT_sb[:, i, c, :],
                        start=(c == 0),
                        stop=(c == NCB - 1),
                    )
                qkS = work.tile([128, CHUNK], BF, tag="qkS", name=f"qkS{i}")
                nc.vector.tensor_copy(out=qkS[:], in_=qk_ps[:])
                rot_ps = ps_scratch.tile([128, CHUNK], F32, tag="scr", name=f"rot{i}")
                nc.tensor.matmul(rot_ps[:], r2_sb[:], qkS[:], start=True, stop=True)
                rot_tiles[i] = (qkS, rot_ps)

            def phase_a_rope(i):
                tsl = ts(i, CHUNK)
                qkS, rot_ps = rot_tiles[i]
                tmp1 = work.tile([128, CHUNK], BF, tag="tmp1", name=f"t1_{i}")
                nc.vector.tensor_mul(tmp1[:], qkS[:], coscos_sb[:, tsl])
                tmp2 = work.tile([128, CHUNK], BF, tag="tmp2", name=f"t2_{i}")
                nc.vector.tensor_mul(tmp2[:], rot_ps[:], sinsin_sb[:, tsl])
                nc.vector.tensor_add(qkrope[:, tsl], tmp1[:], tmp2[:])
                # partition-swapped copy [k';q'] (PE permutation matmul) so
                # score MMs can run in CONCURRENT PE row groups
                sw_ps = ps_scratch.tile([128, CHUNK], F32, tag="scr", name=f"sw{i}")
                nc.tensor.matmul(
                    sw_ps[:], pswap_sb[:], qkrope[:, tsl], start=True, stop=True
                )
                nc.vector.tensor_copy(out=swapped[:, tsl], in_=sw_ps[:])

            def phase_v(g):
                # v-projection for chunk pair (2g, 2g+1), col-tiled pairs
                # interleaved so both column groups stream concurrently
                i = 2 * g + 1
                v_ps = ps_scratch.tile([128, CHUNK], F32, tag="scr", name=f"v{g}")
                for c in range(NCB):
                    nc.tensor.matmul(
                        v_ps[0:H, :],
                        wv_sb[:, c, :],
                        xT_sb[:, i - 1, c, :],
                        start=(c == 0),
                        stop=(c == NCB - 1),
                        skip_group_check=True,
                    )
                    nc.tensor.matmul(
                        v_ps[H:128, :],
                        wv_sb[:, c, :],
                        xT_sb[:, i, c, :],
                        start=(c == 0),
                        stop=(c == NCB - 1),
                        skip_group_check=True,
                    )
                nc.vector.tensor_copy(out=vT_sb[:, g, :], in_=v_ps[:])

            def phase_b(g):
                # transpose 8 s-blocks of the vT pair group g (chunks 2g, 2g+1)
                for half_ in range(2):
                    vn_ps = ps_out.tile(
                        [128, 4, H], BF, tag="out", name=f"vn{g}_{half_}"
                    )
                    base = H * half_
                    for j in range(4):
                        nc.tensor.transpose(
                            vn_ps[:, j, :],
                            vT_sb[base : base + H, g, ts(j, 128)],
                            identb_sb[base : base + H, :],
                        )
                    first = 8 * g + 4 * half_
                    nc.vector.tensor_copy(
                        out=vnat[:, first : first + 4, 0:H], in_=vn_ps[:]
                    )

            out_tiles = {}

            def phase_c_accum(i):
                nsb = 4 * i + 4  # s-blocks 0 .. 4i+3 (0..4i-1 full, then diag)
                out_ps = ps_out.tile([H + 1, CHUNK], F32, tag="out", name=f"o{i}")
                out_tiles[i] = out_ps

                # units: full pairs then diagonal dual-pairs; software-
                # pipelined so scores(u+1) runs on PE during exp(u) on ACT.
                # each pair's two K=64 score MMs run CONCURRENTLY in PE row
                # groups 0:64 (k' from swapped) and 64:128 (k' from qkrope).
                units = [("pair", p) for p in range(2 * i)] + [
                    ("diag", j) for j in (0, 2)
                ]
                staged = []  # (kind, idx, sc_tile)

                def emit_scores(u):
                    kind, idx = u
                    if kind == "pair":
                        sc2 = ps_sc.tile(
                            [128, 2, CHUNK], F32, tag="sc", name=f"s{i}_{idx}"
                        )
                        sb = 2 * idx
                        nc.tensor.matmul(
                            sc2[:, 0, :],
                            swapped[0:H, ts(sb, 128)],
                            qkrope[0:H, ts(i, CHUNK)],
                            start=True,
                            stop=True,
                            skip_group_check=True,
                        )
                        nc.tensor.matmul(
                            sc2[:, 1, :],
                            qkrope[H:128, ts(sb + 1, 128)],
                            swapped[H:128, ts(i, CHUNK)],
                            start=True,
                            stop=True,
                            skip_group_check=True,
                        )
                    else:
                        j = idx  # diag dual-pair (j, j+1)
                        sc2 = ps_sc.tile(
                            [128, 2, CHUNK], F32, tag="sc", name=f"sd{i}_{j}"
                        )
                        lo0 = 128 * j
                        lo1 = 128 * (j + 1)
                        nc.tensor.matmul(
                            sc2[:, 0, lo0:CHUNK],
                            swapped[0:H, ts(4 * i + j, 128)],
                            qkrope[0:H, i * CHUNK + lo0 : (i + 1) * CHUNK],
                            start=True,
                            stop=True,
                            skip_group_check=True,
                        )
                        nc.tensor.matmul(
                            sc2[:, 1, lo1:CHUNK],
                            qkrope[H:128, ts(4 * i + j + 1, 128)],
                            swapped[H:128, i * CHUNK + lo1 : (i + 1) * CHUNK],
                            start=True,
                            stop=True,
                            skip_group_check=True,
                        )
                    staged.append((kind, idx, sc2))

                def emit_exp_num(stage):
                    kind, idx, sc2 = stage
                    if kind == "pair":
                        pexp2 = pexpp.tile(
                            [128, 2, CHUNK], BF, tag="pexp", name=f"p{i}_{idx}"
                        )
                        nc.scalar.activation(
                            out=pexp2[:], in_=sc2[:], func=Exp, scale=0.125
                        )
                        for h_ in range(2):
                            sb = 2 * idx + h_
                            nc.tensor.matmul(
                                out_ps[:],
                                vnat[:, sb, :],
                                pexp2[:, h_, :],
                                start=(sb == 0),
                                stop=False,
                            )
                    else:
                        j = idx
                        pexp2 = pexpp.tile(
                            [128, 2, CHUNK], BF, tag="pexp", name=f"pd{i}_{j}"
                        )
                        for h_ in range(2):
                            jj = j + h_
                            sb = 4 * i + jj
                            lo = 128 * jj
                            nc.scalar.activation(
                                out=pexp2[:, h_, lo:CHUNK],
                                in_=sc2[:, h_, lo:CHUNK],
                                func=Exp,
                                scale=0.125,
                            )
                            nc.vector.tensor_mul(
                                pexp2[:, h_, lo : lo + 128],
                                pexp2[:, h_, lo : lo + 128],
                                trimask_sb[:],
                            )
                            nc.tensor.matmul(
                                out_ps[:, lo:CHUNK],
                                vnat[:, sb, :],
                                pexp2[:, h_, lo:CHUNK],
                                start=(sb == 0),
                                stop=(sb == nsb - 1),
                            )

                emit_scores(units[0])
                for u in units[1:]:
                    emit_scores(u)
                    emit_exp_num(staged.pop(0))
                emit_exp_num(staged.pop(0))

            def phase_c_out(i):
                # evict [num;den] to bf16 and ship; host divides + transposes
                out_ps = out_tiles[i]
                outb = work.tile([H + 1, CHUNK], BF, tag="outb", name=f"ob{i}")
                nc.vector.tensor_copy(out=outb[:], in_=out_ps[:])
                nc.sync.dma_start(out=out_d.ap()[i], in_=outb[:])

            # emission order == static per-engine FIFO order. Keep long-dep
            # ops (rope needs cos/sin; v needs wv+xT1) from blocking
            # independent work, and emit phase-C scores early so ACT never
            # starves once it gets going.
            phase_a_proj(0)
            phase_a_proj(1)
            phase_a_rope(0)
            phase_a_rope(1)
            phase_v(0)
            phase_b(0)
            phase_c_accum(0)
            phase_a_proj(2)
            phase_a_rope(2)
            phase_c_out(0)
            phase_c_accum(1)
            phase_a_proj(3)
            phase_a_rope(3)
            phase_v(1)
            phase_b(1)
            phase_c_out(1)
            phase_c_accum(2)
            phase_c_out(2)
            phase_c_accum(3)
            phase_c_out(3)

    nc.compile()
    return nc


_NC_CACHE = None


def _get_nc():
    global _NC_CACHE
    if _NC_CACHE is None:
        _NC_CACHE = _build_bass()
    return _NC_CACHE


def make_in_maps(x, Wq, Wk, Wv):
    """Host-side prep: shard over batch + precompute constants."""
    coscos, sinsin, r2, trimask, identb2, pswap = _build_consts()
    wqk = np.concatenate([Wq, Wk], axis=1).astype(bf16)  # (C, 128)
    wv = Wv.astype(bf16)
    wqkp = np.ascontiguousarray(wqk.reshape(NCB, 128, 128).transpose(1, 0, 2))
    wvp = np.ascontiguousarray(wv.reshape(NCB, 128, H).transpose(1, 0, 2))
    in_maps = []
    for b in range(B):
        xT = x[b].T.astype(bf16)  # (C, T)
        xTp = np.ascontiguousarray(
            xT.reshape(NCB, 128, NCHUNK, CHUNK).transpose(1, 2, 0, 3)
        )
        in_maps.append(
            {
                "xTp": xTp,
                "wqkp": wqkp,
                "wvp": wvp,
                "r2": r2,
                "coscos": coscos,
                "sinsin": sinsin,
                "trimask": trimask,
                "identb2": identb2,
                "pswap": pswap,
            }
        )
    return in_maps


def finalize_out(raw):
    """raw: (NCHUNK, H+1, CHUNK) bf16 [num;den] -> (T, H) f32 normalized."""
    r = np.asarray(raw, dtype=np.float32)
    num = r[:, 0:H, :]          # (4, 64, 512)
    den = r[:, H, :]            # (4, 512)
    out = num / den[:, None, :]  # (4, 64, 512)
    return np.ascontiguousarray(out.transpose(0, 2, 1).reshape(T, H))


def kernel(x, Wq, Wk, Wv):
    from concourse.bass_utils import run_bass_kernel_spmd

    x = np.asarray(x, dtype=np.float32)
    Wq = np.asarray(Wq, dtype=np.float32)
    Wk = np.asarray(Wk, dtype=np.float32)
    Wv = np.asarray(Wv, dtype=np.float32)

    nc = _get_nc()
    in_maps = make_in_maps(x, Wq, Wk, Wv)
    res = run_bass_kernel_spmd(nc, in_maps, core_ids=list(range(NCORES)))
    out = np.stack([finalize_out(r["outp"]) for r in res.results])  # (B, T, H)
    return np.ascontiguousarray(out.astype(np.float32))


# revision 18
# speedup vs baseline: 1.0350x; 1.0043x over previous
"""Single-head causal attention with RoPE on 8 Trainium2 NeuronCores.

Problem: x:(8,2048,1024), Wq/Wk/Wv:(1024,64) -> out:(8,2048,64)
  q = rope(x@Wq); k = rope(x@Wk); v = x@Wv
  out = softmax(causal(q k^T / sqrt(64))) @ v

Sharding: data-parallel over batch B=8, one batch element per core.

Per-core dataflow (transposed [feature, time] layout so the softmax
reduction lands on the PSUM partition dim via a ones-column matmul):
  A(i): qkT = [Wq|Wk]^T @ xT_chunk (PE, M=128); rot = R2 @ bf16(qkT) (PE)
        q'k' = qkT*coscos + rot*sinsin (DVE) -> qkrope bf16 (q rows 0:64,
        k rows 64:128); swapped[k';q'] partition copies via SBUF-SBUF DMA
        v-proj for chunk PAIRS col-tiled: two chunks run CONCURRENTLY in
        PE column groups 0:64 / 64:128
  B(g): v natural via PE transpose -> vnat[128,16,65] (ones col 64)
  C(i): s-blocks in PAIRS: two K=64 score MMs run CONCURRENTLY in PE row
        groups 0:64 / 64:128 (tile_position from base partitions); ONE
        exp ACTIVATE covers both; diagonal blocks paired + causal-trimmed
        [num;den] psum += vnat_sb^T @ pexp (PE, M=65; ones col -> den)
        evict [num;den] to bf16 and DMA out; normalization (num/den and
        the final [h,t]->[t,h] transpose) happens on HOST after gather
  DMA: x-stream has priority on BOTH HWDGE queues (sync + scalar);
  cos/sin loaded as 64 rows and duplicated on-chip; small consts +
  partition dups on gpsimd (SWDGE).
"""

import numpy as np
import ml_dtypes

B, T, C, H = 8, 2048, 1024, 64
NCORES = 8
CHUNK = 512
NCHUNK = T // CHUNK  # 4
NSB = T // 128       # 16 s-blocks
NCB = C // 128       # 8 c-blocks

bf16 = ml_dtypes.bfloat16


# ---------------------------------------------------------------- host consts
def _build_consts():
    half = H // 2
    inv_freq = (1.0 / (10000.0 ** (np.arange(half, dtype=np.float32) / half))).astype(
        np.float32
    )
    t = np.arange(T, dtype=np.float32)
    freqs = t[:, None] * inv_freq[None, :]  # (T, half) fp32
    cos = np.repeat(np.cos(freqs), 2, axis=-1)  # (T, H)
    sin = np.repeat(np.sin(freqs), 2, axis=-1)
    cosT = np.ascontiguousarray(cos.T).astype(bf16)  # (H, T)
    sinT = np.ascontiguousarray(sin.T).astype(bf16)

    # rot = R @ q with rot[2i] = -q[2i+1], rot[2i+1] = q[2i]
    Rm = np.zeros((H, H), np.float32)
    for i in range(half):
        Rm[2 * i, 2 * i + 1] = -1.0
        Rm[2 * i + 1, 2 * i] = 1.0
    r2 = np.zeros((128, 128), np.float32)
    r2[0:H, 0:H] = Rm.T
    r2[H:128, H:128] = Rm.T
    r2 = r2.astype(bf16)

    sl = np.arange(128)
    trimask = (sl[:, None] <= sl[None, :]).astype(bf16)  # (128, 128)

    identb2 = np.concatenate([np.eye(H), np.eye(H)], axis=0).astype(bf16)  # (128, 64)

    # half-swap permutation: swapped[p] = x[(p+64) % 128]; symmetric
    pswap = np.zeros((128, 128), np.float32)
    for p in range(128):
        pswap[p, (p + 64) % 128] = 1.0
    pswap = pswap.astype(bf16)

    coscos = np.ascontiguousarray(np.concatenate([cosT, cosT], axis=0))  # (128, T)
    sinsin = np.ascontiguousarray(np.concatenate([sinT, sinT], axis=0))

    return coscos, sinsin, r2, trimask, identb2, pswap


# ---------------------------------------------------------------- bass program
def _build_bass():
    import concourse.mybir as mybir
    import concourse.tile as tile
    from concourse import bacc
    from concourse.bass import ts

    BF = mybir.dt.bfloat16
    F32 = mybir.dt.float32
    Exp = mybir.ActivationFunctionType.Exp

    nc = bacc.Bacc(
        "TRN2",
        target_bir_lowering=False,
        debug=False,
        enable_asserts=False,
        num_devices=NCORES,
    )

    # xT prepacked on host to SBUF layout [128(p), chunk, cblk, 512]
    xT_d = nc.dram_tensor("xTp", [128, NCHUNK, NCB, CHUNK], BF, kind="ExternalInput")
    wqk_d = nc.dram_tensor("wqkp", [128, NCB, 128], BF, kind="ExternalInput")
    wv_d = nc.dram_tensor("wvp", [128, NCB, H], BF, kind="ExternalInput")
    r2_d = nc.dram_tensor("r2", [128, 128], BF, kind="ExternalInput")
    coscos_d = nc.dram_tensor("coscos", [128, T], BF, kind="ExternalInput")
    sinsin_d = nc.dram_tensor("sinsin", [128, T], BF, kind="ExternalInput")
    trimask_d = nc.dram_tensor("trimask", [128, 128], BF, kind="ExternalInput")
    identb_d = nc.dram_tensor("identb2", [128, H], BF, kind="ExternalInput")
    pswap_d = nc.dram_tensor("pswap", [128, 128], BF, kind="ExternalInput")
    out_d = nc.dram_tensor("outp", [NCHUNK, H + 1, CHUNK], BF, kind="ExternalOutput")

    with tile.TileContext(nc) as tc:
        with (
            tc.tile_pool(name="persist", bufs=1) as persist,
            tc.tile_pool(name="work", bufs=3) as work,
            tc.tile_pool(name="pexpp", bufs=5) as pexpp,
            tc.tile_pool(name="ps_scratch", bufs=2, space="PSUM") as ps_scratch,
            tc.tile_pool(name="ps_sc", bufs=2, space="PSUM") as ps_sc,
            tc.tile_pool(name="ps_out", bufs=2, space="PSUM") as ps_out,
        ):
            # ---- persistent SBUF tensors
            wqk_sb = persist.tile([128, NCB, 128], BF)
            wv_sb = persist.tile([128, NCB, H], BF)
            r2_sb = persist.tile([128, 128], BF)
            coscos_sb = persist.tile([128, T], BF)
            sinsin_sb = persist.tile([128, T], BF)
            trimask_sb = persist.tile([128, 128], BF)
            identb_sb = persist.tile([128, H], BF)
            pswap_sb = persist.tile([128, 128], BF)
            xT_sb = persist.tile([128, NCHUNK, NCB, CHUNK], BF)
            qkrope = persist.tile([128, T], BF)   # q' rows 0:64, k' rows 64:128
            swapped = persist.tile([128, T], BF)  # k' rows 0:64, q' rows 64:128
            vT_sb = persist.tile([128, NCHUNK // 2, CHUNK], BF)  # stacked pairs
            vnat = persist.tile([128, NSB, H + 1], BF)

            # ---- sync HWDGE queue: wqk (gates first matmul), then x stream.
            # chunk 0 is split across BOTH hwdge queues so projections can
            # start as early as possible; consts ordered by need-time.
            # ---- sync HWDGE queue (fast): ALL bulk input in strict
            # need-order. xT0 split per c-block-pair so projections start
            # as soon as pieces land; cos/sin split per-chunk so rope(0)
            # isn't gated on the full tables. Scalar HWDGE queue measured
            # ~4x slower -- only small consts go there.
            nc.sync.dma_start(out=wqk_sb[:], in_=wqk_d.ap())
            nc.sync.dma_start(out=xT_sb[:, 0], in_=xT_d.ap()[:, 0])
            nc.sync.dma_start(out=coscos_sb[:, 0:CHUNK], in_=coscos_d.ap()[:, 0:CHUNK])
            nc.sync.dma_start(out=sinsin_sb[:, 0:CHUNK], in_=sinsin_d.ap()[:, 0:CHUNK])
            nc.sync.dma_start(out=xT_sb[:, 1], in_=xT_d.ap()[:, 1])
            nc.sync.dma_start(out=coscos_sb[:, CHUNK:T], in_=coscos_d.ap()[:, CHUNK:T])
            nc.sync.dma_start(out=sinsin_sb[:, CHUNK:T], in_=sinsin_d.ap()[:, CHUNK:T])
            nc.sync.dma_start(out=xT_sb[:, 2], in_=xT_d.ap()[:, 2])
            # ---- scalar HWDGE queue (slow ~4x): small early consts + the
            # last-needed x chunk, freeing ~1MB from the fast queue
            nc.scalar.dma_start(out=r2_sb[:], in_=r2_d.ap())
            nc.scalar.dma_start(out=wv_sb[:], in_=wv_d.ap())
            nc.scalar.dma_start(out=pswap_sb[:], in_=pswap_d.ap())
            nc.scalar.dma_start(out=identb_sb[:], in_=identb_d.ap())
            nc.scalar.dma_start(out=trimask_sb[:], in_=trimask_d.ap())
            nc.scalar.dma_start(out=xT_sb[:, 3], in_=xT_d.ap()[:, 3])

            nc.vector.memset(vnat[:], 1.0)  # ones col (64); cols 0:64 overwritten

            # PE warmup: junk matmuls on a zeroed tile bridge the preamble/
            # DMA window so the HAM clock-gate opens before real projections
            zwarm = persist.tile([128, CHUNK], BF)
            nc.vector.memset(zwarm[:], 0.0)
            warm_ps = ps_sc.tile([128, 2, CHUNK], F32, tag="sc", name="warm")
            for w in range(10):
                nc.tensor.matmul(
                    warm_ps[:, 0, :],
                    zwarm[:, 0:128],
                    zwarm[:],
                    start=(w == 0),
                    stop=(w == 9),
                )

            rot_tiles = {}

            def phase_a_proj(i):
                qk_ps = ps_scratch.tile([128, CHUNK], F32, tag="scr", name=f"qk{i}")
                for c in range(NCB):
                    nc.tensor.matmul(
                        qk_ps[:],
                        wqk_sb[:, c, :],
                        xT_sb[:, i, c, :],
                        start=(c == 0),
                        stop=(c == NCB - 1),
                    )
                qkS = work.tile([128, CHUNK], BF, tag="qkS", name=f"qkS{i}")
                nc.vector.tensor_copy(out=qkS[:], in_=qk_ps[:])
                rot_ps = ps_scratch.tile([128, CHUNK], F32, tag="scr", name=f"rot{i}")
                nc.tensor.matmul(rot_ps[:], r2_sb[:], qkS[:], start=True, stop=True)
                rot_tiles[i] = (qkS, rot_ps)

            def phase_a_rope(i):
                tsl = ts(i, CHUNK)
                qkS, rot_ps = rot_tiles[i]
                tmp1 = work.tile([128, CHUNK], BF, tag="tmp1", name=f"t1_{i}")
                nc.vector.tensor_mul(tmp1[:], qkS[:], coscos_sb[:, tsl])
                tmp2 = work.tile([128, CHUNK], BF, tag="tmp2", name=f"t2_{i}")
                nc.vector.tensor_mul(tmp2[:], rot_ps[:], sinsin_sb[:, tsl])
                nc.vector.tensor_add(qkrope[:, tsl], tmp1[:], tmp2[:])
                # partition-swapped copy [k';q'] (PE permutation matmul) so
                # score MMs can run in CONCURRENT PE row groups
                sw_ps = ps_scratch.tile([128, CHUNK], F32, tag="scr", name=f"sw{i}")
                nc.tensor.matmul(
                    sw_ps[:], pswap_sb[:], qkrope[:, tsl], start=True, stop=True
                )
                nc.vector.tensor_copy(out=swapped[:, tsl], in_=sw_ps[:])

            def phase_v(g):
                # v-projection for chunk pair (2g, 2g+1), col-tiled pairs
                # interleaved so both column groups stream concurrently
                i = 2 * g + 1
                v_ps = ps_scratch.tile([128, CHUNK], F32, tag="scr", name=f"v{g}")
                for c in range(NCB):
                    nc.tensor.matmul(
                        v_ps[0:H, :],
                        wv_sb[:, c, :],
                        xT_sb[:, i - 1, c, :],
                        start=(c == 0),
                        stop=(c == NCB - 1),
                        skip_group_check=True,
                    )
                    nc.tensor.matmul(
                        v_ps[H:128, :],
                        wv_sb[:, c, :],
                        xT_sb[:, i, c, :],
                        start=(c == 0),
                        stop=(c == NCB - 1),
                        skip_group_check=True,
                    )
                nc.vector.tensor_copy(out=vT_sb[:, g, :], in_=v_ps[:])

            def phase_b(g):
                # transpose 8 s-blocks of the vT pair group g (chunks 2g, 2g+1)
                for half_ in range(2):
                    vn_ps = ps_out.tile(
                        [128, 4, H], BF, tag="out", name=f"vn{g}_{half_}"
                    )
                    base = H * half_
                    for j in range(4):
                        nc.tensor.transpose(
                            vn_ps[:, j, :],
                            vT_sb[base : base + H, g, ts(j, 128)],
                            identb_sb[base : base + H, :],
                        )
                    first = 8 * g + 4 * half_
                    nc.vector.tensor_copy(
                        out=vnat[:, first : first + 4, 0:H], in_=vn_ps[:]
                    )

            out_tiles = {}

            def phase_c_accum(i):
                nsb = 4 * i + 4  # s-blocks 0 .. 4i+3 (0..4i-1 full, then diag)
                out_ps = ps_out.tile([H + 1, CHUNK], F32, tag="out", name=f"o{i}")
                out_tiles[i] = out_ps

                # units: full pairs then diagonal dual-pairs; software-
                # pipelined so scores(u+1) runs on PE during exp(u) on ACT.
                # each pair's two K=64 score MMs run CONCURRENTLY in PE row
                # groups 0:64 (k' from swapped) and 64:128 (k' from qkrope).
                units = [("pair", p) for p in range(2 * i)] + [
                    ("diag", j) for j in (0, 2)
                ]
                staged = []  # (kind, idx, sc_tile)

                def emit_scores(u):
                    kind, idx = u
                    if kind == "pair":
                        sc2 = ps_sc.tile(
                            [128, 2, CHUNK], F32, tag="sc", name=f"s{i}_{idx}"
                        )
                        sb = 2 * idx
                        nc.tensor.matmul(
                            sc2[:, 0, :],
                            swapped[0:H, ts(sb, 128)],
                            qkrope[0:H, ts(i, CHUNK)],
                            start=True,
                            stop=True,
                            skip_group_check=True,
                        )
                        nc.tensor.matmul(
                            sc2[:, 1, :],
                            qkrope[H:128, ts(sb + 1, 128)],
                            swapped[H:128, ts(i, CHUNK)],
                            start=True,
                            stop=True,
                            skip_group_check=True,
                        )
                    else:
                        j = idx  # diag dual-pair (j, j+1)
                        sc2 = ps_sc.tile(
                            [128, 2, CHUNK], F32, tag="sc", name=f"sd{i}_{j}"
                        )
                        lo0 = 128 * j
                        lo1 = 128 * (j + 1)
                        nc.tensor.matmul(
                            sc2[:, 0, lo0:CHUNK],
                            swapped[0:H, ts(4 * i + j, 128)],
                            qkrope[0:H, i * CHUNK + lo0 : (i + 1) * CHUNK],
                            start=True,
                            stop=True,
                            skip_group_check=True,
                        )
                        nc.tensor.matmul(
                            sc2[:, 1, lo1:CHUNK],
                            qkrope[H:128, ts(4 * i + j + 1, 128)],
                            swapped[H:128, i * CHUNK + lo1 : (i + 1) * CHUNK],
                            start=True,
                            stop=True,
                            skip_group_check=True,
                        )
                    staged.append((kind, idx, sc2))

                def emit_exp_num(stage):
                    kind, idx, sc2 = stage
                    if kind == "pair":
                        pexp2 = pexpp.tile(
                            [128, 2, CHUNK], BF, tag="pexp", name=f"p{i}_{idx}"
                        )
                        nc.scalar.activation(
                            out=pexp2[:], in_=sc2[:], func=Exp, scale=0.125
                        )
                        for h_ in range(2):
                            sb = 2 * idx + h_
                            nc.tensor.matmul(
                                out_ps[:],
                                vnat[:, sb, :],
                                pexp2[:, h_, :],
                                start=(sb == 0),
                                stop=False,
                            )
                    else:
                        j = idx
                        pexp2 = pexpp.tile(
                            [128, 2, CHUNK], BF, tag="pexp", name=f"pd{i}_{j}"
                        )
                        for h_ in range(2):
                            jj = j + h_
                            sb = 4 * i + jj
                            lo = 128 * jj
                            nc.scalar.activation(
                                out=pexp2[:, h_, lo:CHUNK],
                                in_=sc2[:, h_, lo:CHUNK],
                                func=Exp,
                                scale=0.125,
                            )
                            nc.vector.tensor_mul(
                                pexp2[:, h_, lo : lo + 128],
                                pexp2[:, h_, lo : lo + 128],
                                trimask_sb[:],
                            )
                            nc.tensor.matmul(
                                out_ps[:, lo:CHUNK],
                                vnat[:, sb, :],
                                pexp2[:, h_, lo:CHUNK],
                                start=(sb == 0),
                                stop=(sb == nsb - 1),
                            )

                emit_scores(units[0])
                for u in units[1:]:
                    emit_scores(u)
                    emit_exp_num(staged.pop(0))
                emit_exp_num(staged.pop(0))

            def phase_c_out(i):
                # evict [num;den] to bf16 and ship; host divides + transposes
                out_ps = out_tiles[i]
                outb = work.tile([H + 1, CHUNK], BF, tag="outb", name=f"ob{i}")
                nc.vector.tensor_copy(out=outb[:], in_=out_ps[:])
                nc.sync.dma_start(out=out_d.ap()[i], in_=outb[:])

            # emission order == static per-engine FIFO order. Keep long-dep
            # ops (rope needs cos/sin; v needs wv+xT1) from blocking
            # independent work, and emit phase-C scores early so ACT never
            # starves once it gets going.
            phase_a_proj(0)
            phase_a_proj(1)
            phase_a_rope(0)
            phase_a_rope(1)
            phase_v(0)
            phase_b(0)
            phase_c_accum(0)
            phase_a_proj(2)
            phase_a_rope(2)
            phase_c_out(0)
            phase_c_accum(1)
            phase_a_proj(3)
            phase_a_rope(3)
            phase_v(1)
            phase_b(1)
            phase_c_out(1)
            phase_c_accum(2)
            phase_c_out(2)
            phase_c_accum(3)
            phase_c_out(3)

    nc.compile()
    return nc


_NC_CACHE = None


def _get_nc():
    global _NC_CACHE
    if _NC_CACHE is None:
        _NC_CACHE = _build_bass()
    return _NC_CACHE


def make_in_maps(x, Wq, Wk, Wv):
    """Host-side prep: shard over batch + precompute constants."""
    coscos, sinsin, r2, trimask, identb2, pswap = _build_consts()
    wqk = np.concatenate([Wq, Wk], axis=1).astype(bf16)  # (C, 128)
    wv = Wv.astype(bf16)
    wqkp = np.ascontiguousarray(wqk.reshape(NCB, 128, 128).transpose(1, 0, 2))
    wvp = np.ascontiguousarray(wv.reshape(NCB, 128, H).transpose(1, 0, 2))
    in_maps = []
    for b in range(B):
        xT = x[b].T.astype(bf16)  # (C, T)
        xTp = np.ascontiguousarray(
            xT.reshape(NCB, 128, NCHUNK, CHUNK).transpose(1, 2, 0, 3)
        )
        in_maps.append(
            {
                "xTp": xTp,
                "wqkp": wqkp,
                "wvp": wvp,
                "r2": r2,
                "coscos": coscos,
                "sinsin": sinsin,
                "trimask": trimask,
                "identb2": identb2,
                "pswap": pswap,
            }
        )
    return in_maps


def finalize_out(raw):
    """raw: (NCHUNK, H+1, CHUNK) bf16 [num;den] -> (T, H) f32 normalized."""
    r = np.asarray(raw, dtype=np.float32)
    num = r[:, 0:H, :]          # (4, 64, 512)
    den = r[:, H, :]            # (4, 512)
    out = num / den[:, None, :]  # (4, 64, 512)
    return np.ascontiguousarray(out.transpose(0, 2, 1).reshape(T, H))


def kernel(x, Wq, Wk, Wv):
    from concourse.bass_utils import run_bass_kernel_spmd

    x = np.asarray(x, dtype=np.float32)
    Wq = np.asarray(Wq, dtype=np.float32)
    Wk = np.asarray(Wk, dtype=np.float32)
    Wv = np.asarray(Wv, dtype=np.float32)

    nc = _get_nc()
    in_maps = make_in_maps(x, Wq, Wk, Wv)
    res = run_bass_kernel_spmd(nc, in_maps, core_ids=list(range(NCORES)))
    out = np.stack([finalize_out(r["outp"]) for r in res.results])  # (B, T, H)
    return np.ascontiguousarray(out.astype(np.float32))


# revision 19
# speedup vs baseline: 1.0722x; 1.0359x over previous
"""Single-head causal attention with RoPE on 8 Trainium2 NeuronCores.

Problem: x:(8,2048,1024), Wq/Wk/Wv:(1024,64) -> out:(8,2048,64)
  q = rope(x@Wq); k = rope(x@Wk); v = x@Wv
  out = softmax(causal(q k^T / sqrt(64))) @ v

Sharding: data-parallel over batch B=8, one batch element per core.

Per-core dataflow (transposed [feature, time] layout so the softmax
reduction lands on the PSUM partition dim via a ones-column matmul):
  A(i): qkT = [Wq|Wk]^T @ xT_chunk (PE, M=128); rot = R2 @ bf16(qkT) (PE)
        q'k' = qkT*cos + rot*sin (DVE) -> qkrope bf16 (q rows 0:64,
        k rows 64:128); swapped[k';q'] via PE permutation matmul
        v-proj for chunk PAIRS col-tiled: two chunks run CONCURRENTLY in
        PE column groups 0:64 / 64:128
  B(g): v natural via PE transpose -> vnat[128,16,65] (ones col 64)
  C(i): s-blocks in PAIRS: two K=64 score MMs run CONCURRENTLY in PE row
        groups 0:64 / 64:128 (tile_position from base partitions); ONE
        exp ACTIVATE covers both; diagonal blocks paired + causal-trimmed
        [num;den] psum += vnat_sb^T @ pexp (PE, M=65; ones col -> den)
        evict [num;den] to bf16 and DMA out; normalization (num/den and
        the final [h,t]->[t,h] transpose) happens on HOST after gather
  DMA: the C0-critical input set [wqk|xT0|cos0|sin0] is packed into ONE
  contiguous 1.5MB "head" transfer (big transfers sidestep the slow
  multi-small-transfer startup); cos/sin for chunks 1-3 packed into one
  "costail"; xT3 + small consts ride the slower scalar HWDGE queue.
"""

import numpy as np
import ml_dtypes

B, T, C, H = 8, 2048, 1024, 64
NCORES = 8
CHUNK = 512
NCHUNK = T // CHUNK  # 4
NSB = T // 128       # 16 s-blocks
NCB = C // 128       # 8 c-blocks

HEAD_N = 1024 + 4096 + 512 + 512  # wqk | xT0 | cos0 | sin0 per partition
TAIL_N = 3072                     # cos chunks 1-3 | sin chunks 1-3

bf16 = ml_dtypes.bfloat16


# ---------------------------------------------------------------- host consts
def _build_consts():
    half = H // 2
    inv_freq = (1.0 / (10000.0 ** (np.arange(half, dtype=np.float32) / half))).astype(
        np.float32
    )
    t = np.arange(T, dtype=np.float32)
    freqs = t[:, None] * inv_freq[None, :]  # (T, half) fp32
    cos = np.repeat(np.cos(freqs), 2, axis=-1)  # (T, H)
    sin = np.repeat(np.sin(freqs), 2, axis=-1)
    cosT = np.ascontiguousarray(cos.T).astype(bf16)  # (H, T)
    sinT = np.ascontiguousarray(sin.T).astype(bf16)

    # rot = R @ q with rot[2i] = -q[2i+1], rot[2i+1] = q[2i]
    Rm = np.zeros((H, H), np.float32)
    for i in range(half):
        Rm[2 * i, 2 * i + 1] = -1.0
        Rm[2 * i + 1, 2 * i] = 1.0
    r2 = np.zeros((128, 128), np.float32)
    r2[0:H, 0:H] = Rm.T
    r2[H:128, H:128] = Rm.T
    r2 = r2.astype(bf16)

    sl = np.arange(128)
    trimask = (sl[:, None] <= sl[None, :]).astype(bf16)  # (128, 128)

    identb2 = np.concatenate([np.eye(H), np.eye(H)], axis=0).astype(bf16)  # (128, 64)

    # half-swap permutation: swapped[p] = x[(p+64) % 128]; symmetric
    pswap = np.zeros((128, 128), np.float32)
    for p in range(128):
        pswap[p, (p + 64) % 128] = 1.0
    pswap = pswap.astype(bf16)

    coscos = np.ascontiguousarray(np.concatenate([cosT, cosT], axis=0))  # (128, T)
    sinsin = np.ascontiguousarray(np.concatenate([sinT, sinT], axis=0))

    return coscos, sinsin, r2, trimask, identb2, pswap


# ---------------------------------------------------------------- bass program
def _build_bass():
    import concourse.mybir as mybir
    import concourse.tile as tile
    from concourse import bacc
    from concourse.bass import ts

    BF = mybir.dt.bfloat16
    F32 = mybir.dt.float32
    Exp = mybir.ActivationFunctionType.Exp

    nc = bacc.Bacc(
        "TRN2",
        target_bir_lowering=False,
        debug=False,
        enable_asserts=False,
        num_devices=NCORES,
    )

    # xT prepacked on host to SBUF layout [128(p), chunk, cblk, 512]
    head_d = nc.dram_tensor("headp", [128, HEAD_N], BF, kind="ExternalInput")
    tail_d = nc.dram_tensor("tailp", [128, TAIL_N], BF, kind="ExternalInput")
    xT_d = nc.dram_tensor("xTp", [128, NCHUNK, NCB, CHUNK], BF, kind="ExternalInput")
    wv_d = nc.dram_tensor("wvp", [128, NCB, H], BF, kind="ExternalInput")
    r2_d = nc.dram_tensor("r2", [128, 128], BF, kind="ExternalInput")
    trimask_d = nc.dram_tensor("trimask", [128, 128], BF, kind="ExternalInput")
    identb_d = nc.dram_tensor("identb2", [128, H], BF, kind="ExternalInput")
    pswap_d = nc.dram_tensor("pswap", [128, 128], BF, kind="ExternalInput")
    out_d = nc.dram_tensor("outp", [NCHUNK, H + 1, CHUNK], BF, kind="ExternalOutput")

    with tile.TileContext(nc) as tc:
        with (
            tc.tile_pool(name="persist", bufs=1) as persist,
            tc.tile_pool(name="work", bufs=3) as work,
            tc.tile_pool(name="pexpp", bufs=5) as pexpp,
            tc.tile_pool(name="ps_scratch", bufs=2, space="PSUM") as ps_scratch,
            tc.tile_pool(name="ps_sc", bufs=2, space="PSUM") as ps_sc,
            tc.tile_pool(name="ps_out", bufs=2, space="PSUM") as ps_out,
        ):
            # ---- persistent SBUF tensors
            head_sb = persist.tile([128, HEAD_N], BF)
            tail_sb = persist.tile([128, TAIL_N], BF)
            xT_sb = persist.tile([128, NCHUNK - 1, NCB, CHUNK], BF)  # chunks 1-3
            wv_sb = persist.tile([128, NCB, H], BF)
            r2_sb = persist.tile([128, 128], BF)
            trimask_sb = persist.tile([128, 128], BF)
            identb_sb = persist.tile([128, H], BF)
            pswap_sb = persist.tile([128, 128], BF)
            qkrope = persist.tile([128, T], BF)   # q' rows 0:64, k' rows 64:128
            swapped = persist.tile([128, T], BF)  # k' rows 0:64, q' rows 64:128
            vT_sb = persist.tile([128, NCHUNK // 2, CHUNK], BF)  # stacked pairs
            vnat = persist.tile([128, NSB, H + 1], BF)

            # view helpers into the packed head/tail tiles
            def wqk_v(c):
                return head_sb[:, c * 128 : (c + 1) * 128]

            def xT_v(i, c):
                if i == 0:
                    return head_sb[:, 1024 + c * CHUNK : 1024 + (c + 1) * CHUNK]
                return xT_sb[:, i - 1, c, :]

            def cos_v(i):
                if i == 0:
                    return head_sb[:, 5120:5632]
                return tail_sb[:, (i - 1) * CHUNK : i * CHUNK]

            def sin_v(i):
                if i == 0:
                    return head_sb[:, 5632:6144]
                return tail_sb[:, 1536 + (i - 1) * CHUNK : 1536 + i * CHUNK]

            # ---- sync HWDGE queue (fast): bulk input, strict need-order,
            # biggest-possible transfers
            nc.sync.dma_start(out=head_sb[:], in_=head_d.ap())
            nc.sync.dma_start(out=xT_sb[:, 0], in_=xT_d.ap()[:, 1])
            nc.sync.dma_start(out=tail_sb[:], in_=tail_d.ap())
            nc.sync.dma_start(out=xT_sb[:, 1], in_=xT_d.ap()[:, 2])
            # ---- scalar HWDGE queue (slow ~4x): small early consts + the
            # last-needed x chunk
            nc.scalar.dma_start(out=r2_sb[:], in_=r2_d.ap())
            nc.scalar.dma_start(out=wv_sb[:], in_=wv_d.ap())
            nc.scalar.dma_start(out=pswap_sb[:], in_=pswap_d.ap())
            nc.scalar.dma_start(out=identb_sb[:], in_=identb_d.ap())
            nc.scalar.dma_start(out=trimask_sb[:], in_=trimask_d.ap())
            nc.scalar.dma_start(out=xT_sb[:, 2], in_=xT_d.ap()[:, 3])

            nc.vector.memset(vnat[:], 1.0)  # ones col (64); cols 0:64 overwritten

            # PE warmup: junk matmuls on a zeroed tile bridge the preamble/
            # DMA window so the HAM clock-gate opens before real projections
            zwarm = persist.tile([128, CHUNK], BF)
            nc.vector.memset(zwarm[:], 0.0)
            warm_ps = ps_sc.tile([128, 2, CHUNK], F32, tag="sc", name="warm")
            for w in range(10):
                nc.tensor.matmul(
                    warm_ps[:, 0, :],
                    zwarm[:, 0:128],
                    zwarm[:],
                    start=(w == 0),
                    stop=(w == 9),
                )

            rot_tiles = {}

            def phase_a_proj(i):
                qk_ps = ps_scratch.tile([128, CHUNK], F32, tag="scr", name=f"qk{i}")
                for c in range(NCB):
                    nc.tensor.matmul(
                        qk_ps[:],
                        wqk_v(c),
                        xT_v(i, c),
                        start=(c == 0),
                        stop=(c == NCB - 1),
                    )
                qkS = work.tile([128, CHUNK], BF, tag="qkS", name=f"qkS{i}")
                nc.vector.tensor_copy(out=qkS[:], in_=qk_ps[:])
                rot_ps = ps_scratch.tile([128, CHUNK], F32, tag="scr", name=f"rot{i}")
                nc.tensor.matmul(rot_ps[:], r2_sb[:], qkS[:], start=True, stop=True)
                rot_tiles[i] = (qkS, rot_ps)

            def phase_a_rope(i):
                tsl = ts(i, CHUNK)
                qkS, rot_ps = rot_tiles[i]
                tmp1 = work.tile([128, CHUNK], BF, tag="tmp1", name=f"t1_{i}")
                nc.vector.tensor_mul(tmp1[:], qkS[:], cos_v(i))
                tmp2 = work.tile([128, CHUNK], BF, tag="tmp2", name=f"t2_{i}")
                nc.vector.tensor_mul(tmp2[:], rot_ps[:], sin_v(i))
                nc.vector.tensor_add(qkrope[:, tsl], tmp1[:], tmp2[:])
                # partition-swapped copy [k';q'] (PE permutation matmul) so
                # score MMs can run in CONCURRENT PE row groups
                sw_ps = ps_scratch.tile([128, CHUNK], F32, tag="scr", name=f"sw{i}")
                nc.tensor.matmul(
                    sw_ps[:], pswap_sb[:], qkrope[:, tsl], start=True, stop=True
                )
                nc.vector.tensor_copy(out=swapped[:, tsl], in_=sw_ps[:])

            def phase_v(g):
                # v-projection for chunk pair (2g, 2g+1), col-tiled pairs
                # interleaved so both column groups stream concurrently
                i = 2 * g + 1
                v_ps = ps_scratch.tile([128, CHUNK], F32, tag="scr", name=f"v{g}")
                for c in range(NCB):
                    nc.tensor.matmul(
                        v_ps[0:H, :],
                        wv_sb[:, c, :],
                        xT_v(i - 1, c),
                        start=(c == 0),
                        stop=(c == NCB - 1),
                        skip_group_check=True,
                    )
                    nc.tensor.matmul(
                        v_ps[H:128, :],
                        wv_sb[:, c, :],
                        xT_v(i, c),
                        start=(c == 0),
                        stop=(c == NCB - 1),
                        skip_group_check=True,
                    )
                nc.vector.tensor_copy(out=vT_sb[:, g, :], in_=v_ps[:])

            def phase_b(g):
                # transpose 8 s-blocks of the vT pair group g (chunks 2g, 2g+1)
                for half_ in range(2):
                    vn_ps = ps_out.tile(
                        [128, 4, H], BF, tag="out", name=f"vn{g}_{half_}"
                    )
                    base = H * half_
                    for j in range(4):
                        nc.tensor.transpose(
                            vn_ps[:, j, :],
                            vT_sb[base : base + H, g, ts(j, 128)],
                            identb_sb[base : base + H, :],
                        )
                    first = 8 * g + 4 * half_
                    nc.vector.tensor_copy(
                        out=vnat[:, first : first + 4, 0:H], in_=vn_ps[:]
                    )

            out_tiles = {}

            def phase_c_accum(i):
                nsb = 4 * i + 4  # s-blocks 0 .. 4i+3 (0..4i-1 full, then diag)
                out_ps = ps_out.tile([H + 1, CHUNK], F32, tag="out", name=f"o{i}")
                out_tiles[i] = out_ps

                # units: full pairs then diagonal dual-pairs; software-
                # pipelined so scores(u+1) runs on PE during exp(u) on ACT.
                # each pair's two K=64 score MMs run CONCURRENTLY in PE row
                # groups 0:64 (k' from swapped) and 64:128 (k' from qkrope).
                units = [("pair", p) for p in range(2 * i)] + [
                    ("diag", j) for j in (0, 2)
                ]
                staged = []  # (kind, idx, sc_tile)

                def emit_scores(u):
                    kind, idx = u
                    if kind == "pair":
                        sc2 = ps_sc.tile(
                            [128, 2, CHUNK], F32, tag="sc", name=f"s{i}_{idx}"
                        )
                        sb = 2 * idx
                        nc.tensor.matmul(
                            sc2[:, 0, :],
                            swapped[0:H, ts(sb, 128)],
                            qkrope[0:H, ts(i, CHUNK)],
                            start=True,
                            stop=True,
                            skip_group_check=True,
                        )
                        nc.tensor.matmul(
                            sc2[:, 1, :],
                            qkrope[H:128, ts(sb + 1, 128)],
                            swapped[H:128, ts(i, CHUNK)],
                            start=True,
                            stop=True,
                            skip_group_check=True,
                        )
                    else:
                        j = idx  # diag dual-pair (j, j+1)
                        sc2 = ps_sc.tile(
                            [128, 2, CHUNK], F32, tag="sc", name=f"sd{i}_{j}"
                        )
                        lo0 = 128 * j
                        lo1 = 128 * (j + 1)
                        nc.tensor.matmul(
                            sc2[:, 0, lo0:CHUNK],
                            swapped[0:H, ts(4 * i + j, 128)],
                            qkrope[0:H, i * CHUNK + lo0 : (i + 1) * CHUNK],
                            start=True,
                            stop=True,
                            skip_group_check=True,
                        )
                        nc.tensor.matmul(
                            sc2[:, 1, lo1:CHUNK],
                            qkrope[H:128, ts(4 * i + j + 1, 128)],
                            swapped[H:128, i * CHUNK + lo1 : (i + 1) * CHUNK],
                            start=True,
                            stop=True,
                            skip_group_check=True,
                        )
                    staged.append((kind, idx, sc2))

                def emit_exp_num(stage):
                    kind, idx, sc2 = stage
                    if kind == "pair":
                        pexp2 = pexpp.tile(
                            [128, 2, CHUNK], BF, tag="pexp", name=f"p{i}_{idx}"
                        )
                        nc.scalar.activation(
                            out=pexp2[:], in_=sc2[:], func=Exp, scale=0.125
                        )
                        for h_ in range(2):
                            sb = 2 * idx + h_
                            nc.tensor.matmul(
                                out_ps[:],
                                vnat[:, sb, :],
                                pexp2[:, h_, :],
                                start=(sb == 0),
                                stop=False,
                            )
                    else:
                        j = idx
                        pexp2 = pexpp.tile(
                            [128, 2, CHUNK], BF, tag="pexp", name=f"pd{i}_{j}"
                        )
                        for h_ in range(2):
                            jj = j + h_
                            sb = 4 * i + jj
                            lo = 128 * jj
                            nc.scalar.activation(
                                out=pexp2[:, h_, lo:CHUNK],
                                in_=sc2[:, h_, lo:CHUNK],
                                func=Exp,
                                scale=0.125,
                            )
                            nc.vector.tensor_mul(
                                pexp2[:, h_, lo : lo + 128],
                                pexp2[:, h_, lo : lo + 128],
                                trimask_sb[:],
                            )
                            nc.tensor.matmul(
                                out_ps[:, lo:CHUNK],
                                vnat[:, sb, :],
                                pexp2[:, h_, lo:CHUNK],
                                start=(sb == 0),
                                stop=(sb == nsb - 1),
                            )

                emit_scores(units[0])
                for u in units[1:]:
                    emit_scores(u)
                    emit_exp_num(staged.pop(0))
                emit_exp_num(staged.pop(0))

            def phase_c_out(i):
                # evict [num;den] to bf16 and ship; host divides + transposes
                out_ps = out_tiles[i]
                outb = work.tile([H + 1, CHUNK], BF, tag="outb", name=f"ob{i}")
                nc.vector.tensor_copy(out=outb[:], in_=out_ps[:])
                nc.sync.dma_start(out=out_d.ap()[i], in_=outb[:])

            # emission order == per-engine priority. Keep long-dep ops from
            # blocking independent work; emit phase-C scores early so ACT
            # never starves once it gets going.
            phase_a_proj(0)
            phase_a_rope(0)
            phase_a_proj(1)
            phase_a_rope(1)
            phase_v(0)
            phase_b(0)
            phase_c_accum(0)
            phase_a_proj(2)
            phase_a_rope(2)
            phase_c_out(0)
            phase_c_accum(1)
            phase_a_proj(3)
            phase_a_rope(3)
            phase_v(1)
            phase_b(1)
            phase_c_out(1)
            phase_c_accum(2)
            phase_c_out(2)
            phase_c_accum(3)
            phase_c_out(3)

    nc.compile()
    return nc


_NC_CACHE = None


def _get_nc():
    global _NC_CACHE
    if _NC_CACHE is None:
        _NC_CACHE = _build_bass()
    return _NC_CACHE


def make_in_maps(x, Wq, Wk, Wv):
    """Host-side prep: shard over batch + precompute constants."""
    coscos, sinsin, r2, trimask, identb2, pswap = _build_consts()
    wqk = np.concatenate([Wq, Wk], axis=1).astype(bf16)  # (C, 128)
    wv = Wv.astype(bf16)
    wqkp = np.ascontiguousarray(wqk.reshape(NCB, 128, 128).transpose(1, 0, 2))
    wvp = np.ascontiguousarray(wv.reshape(NCB, 128, H).transpose(1, 0, 2))
    tailp = np.ascontiguousarray(
        np.concatenate([coscos[:, CHUNK:T], sinsin[:, CHUNK:T]], axis=1)
    )
    in_maps = []
    for b in range(B):
        xT = x[b].T.astype(bf16)  # (C, T)
        xTp = np.ascontiguousarray(
            xT.reshape(NCB, 128, NCHUNK, CHUNK).transpose(1, 2, 0, 3)
        )
        headp = np.ascontiguousarray(
            np.concatenate(
                [
                    wqkp.reshape(128, 1024),
                    xTp[:, 0].reshape(128, 4096),
                    coscos[:, 0:CHUNK],
                    sinsin[:, 0:CHUNK],
                ],
                axis=1,
            )
        )
        in_maps.append(
            {
                "headp": headp,
                "tailp": tailp,
                "xTp": xTp,
                "wvp": wvp,
                "r2": r2,
                "trimask": trimask,
                "identb2": identb2,
                "pswap": pswap,
            }
        )
    return in_maps


def finalize_out(raw):
    """raw: (NCHUNK, H+1, CHUNK) bf16 [num;den] -> (T, H) f32 normalized."""
    r = np.asarray(raw, dtype=np.float32)
    num = r[:, 0:H, :]          # (4, 64, 512)
    den = r[:, H, :]            # (4, 512)
    out = num / den[:, None, :]  # (4, 64, 512)
    return np.ascontiguousarray(out.transpose(0, 2, 1).reshape(T, H))


def kernel(x, Wq, Wk, Wv):
    from concourse.bass_utils import run_bass_kernel_spmd

    x = np.asarray(x, dtype=np.float32)
    Wq = np.asarray(Wq, dtype=np.float32)
    Wk = np.asarray(Wk, dtype=np.float32)
    Wv = np.asarray(Wv, dtype=np.float32)

    nc = _get_nc()
    in_maps = make_in_maps(x, Wq, Wk, Wv)
    res = run_bass_kernel_spmd(nc, in_maps, core_ids=list(range(NCORES)))
    out = np.stack([finalize_out(r["outp"]) for r in res.results])  # (B, T, H)
    return np.ascontiguousarray(out.astype(np.float32))
